# revision 13
# baseline (speedup 1.0000x reference)
"""Causal Conv1d (B=8, C=256, T=4096, H=512, K=4) on 8 TRN2 NeuronCores.

Strategy: data-parallel over batch — core i computes batch i.
Per core: out[h, t] = sum_{k, c} W[h, c*K+k] * xpad[c, t+k] + bias[h]
where xpad is x left-padded by K-1 zeros (host side).

Schedule (HW-measured rationale, see probe.py history):
- Weight-stationary accumulating chains: for each weight chunk
  (hj, q=(k,cc)) the loaded stationary operand feeds 2 consecutive
  matmuls into different PSUM banks; chains accumulate over the 8 q
  chunks (start=(q==0)/stop=(q==7)). This makes LDWEIGHTS free.
- Small loop body: per-matmul issue rate degrades sharply once the
  per-iteration PE instruction stream exceeds ~1 IRAM block (64-MM
  bodies issue at ~0.2-0.28 ns/col vs ~0.45 for 256-MM bodies). So
  the For_i body covers ONE t-quarter (1024 t-cols): 4 hj x 8 q x
  2 tj = 64 matmuls, with the t-quarter offset coming from the loop
  variable via register-offset (dynamic) APs on the rhs and the
  DRAM output. ldweights APs must stay static, which is why the
  loop iterates over t only.
- For_i(staggered_reset=True): the default back-edge drains all
  engines (~2-6 us/iteration); staggered reset pipelines iterations.
- bf16 x/W/out (fp32 PSUM accumulation): halves every DMA; matmul
  column rate is dtype-independent. rel err ~3e-3 (gate is 2e-2).
- W + bias load once in the prologue (weight-resident steady state);
  x streams per rep: each iteration reloads its own quarter right
  after its last read, so the reload of rep r+1 overlaps rep r's
  remaining compute. For a single-shot call the reloads rewrite the
  same data the prologue loaded, so the one-shot output is exact.
- PSUM: 8 banks in static roles (hj, tj). Consumption (bias-add via
  DVE/ACT + out-DMA on ACT/SP) of iteration j's banks overlaps
  iteration j+1's matmuls.
"""

import numpy as np

import concourse.bass as bass
import concourse.mybir as mybir
import concourse.tile as tile
from concourse import bacc
from concourse import bass2jax
from concourse.ap import AP

B, C, T = 8, 256, 4096
H, K = 512, 4
PAD = K - 1

N_CORES = 8
TT = 512                # t-tile (free dim per matmul, one fp32 PSUM bank)
N_HCHUNK = H // 128     # 4
N_CCHUNK = C // 128     # 2
N_MM = N_CCHUNK * K     # 8 weight chunks per hj
NQ = 4                  # t-quarters per rep
QT = T // NQ            # 1024 t-cols per quarter
XC = T + PAD            # 4099 resident x cols per cc

_COMPILED = {}


def _build(reps=1, staggered=True):
    f32 = mybir.dt.float32
    bf16 = mybir.dt.bfloat16
    nc = bacc.Bacc("TRN2", target_bir_lowering=False, debug=False)

    x_ext = nc.declare_dram_parameter("x", [N_CCHUNK, 128, XC], bf16, isOutput=False)
    # wt[hj][c, q*128+m]: lhsT for (q=k*N_CCHUNK+cc, h-chunk hj).
    wt_ext = nc.declare_dram_parameter(
        "wt", [N_HCHUNK, 128, N_MM * 128], bf16, isOutput=False
    )
    b_ext = nc.declare_dram_parameter("bias", [128, N_HCHUNK], f32, isOutput=False)
    out_ext = nc.declare_dram_parameter("out", [H, T], bf16, isOutput=True)

    CH = N_MM * 128

    with tile.TileContext(nc) as tc:
        with (
            tc.tile_pool(name="wpool", bufs=1) as wpool,
            tc.tile_pool(name="opool", bufs=8) as opool,
            tc.tile_pool(name="psum", bufs=8, space="PSUM") as psum_pool,
        ):
            # Prologue: resident x (2 tiles [128, 4099]), W chunks, bias.
            xts = []
            for cc in range(N_CCHUNK):
                xt = wpool.tile([128, XC], bf16, name=f"x{cc}")
                nc.sync.dma_start(xt[:], x_ext[cc])
                xts.append(xt)
            wtiles = []
            for hj in range(N_HCHUNK):
                wt = wpool.tile([128, CH], bf16, name=f"w{hj}")
                nc.sync.dma_start(wt[:], wt_ext[hj])
                wtiles.append(wt)
            btile = wpool.tile([128, N_HCHUNK], f32, name="btile")
            nc.sync.dma_start(btile[:], b_ext[:])

            def dyn(base, off):
                if off is None:
                    return base
                return AP(
                    base.tensor,
                    base.offset + off,
                    base.ap,
                    dep_tracking_offset=base.offset,
                )

            def body(toff):
                # toff: t-quarter offset (RuntimeValue elements) or None (=0).
                for hj in range(N_HCHUNK):
                    pss = [
                        psum_pool.tile([128, TT], f32, name="ps", tag="ps")
                        for _ in range(2)
                    ]
                    for q in range(N_MM):
                        k, cc = divmod(q, N_CCHUNK)
                        for tj in range(2):
                            nc.tensor.matmul(
                                pss[tj][:],
                                wtiles[hj][:, q * 128 : q * 128 + 128],
                                dyn(
                                    xts[cc][:, tj * TT + k : tj * TT + k + TT],
                                    toff,
                                ),
                                start=(q == 0),
                                stop=(q == N_MM - 1),
                            )
                    for tj in range(2):
                        ot = opool.tile([128, TT], bf16, name="ot", tag="ot")
                        if tj % 2:
                            nc.scalar.add(ot[:], pss[tj][:], btile[:, hj : hj + 1])
                        else:
                            nc.vector.tensor_scalar_add(
                                ot[:], pss[tj][:], btile[:, hj : hj + 1]
                            )
                        dst = dyn(
                            out_ext[hj * 128 : (hj + 1) * 128, tj * TT : (tj + 1) * TT],
                            toff,
                        )
                        # split output pushes: ACT takes tj1 (it also runs
                        # half the bias-adds), SP takes tj0 (it also runs
                        # the x quarter reloads)
                        eng = nc.scalar if tj % 2 else nc.sync
                        eng.dma_start(dst, ot[:])

                # Reload this quarter's x for the next rep, right after its
                # last read. Quarter j covers x cols [j*1024, j*1024+1024),
                # so every column is rewritten exactly once per rep and read
                # ranges crossing into the next quarter's head stay
                # consistent (one-rep-delayed input pipeline; a single-shot
                # call rewrites identical data).
                for cc in range(N_CCHUNK):
                    if toff is None:
                        nc.sync.dma_start(xts[cc][:, 0:QT], x_ext[cc][:, 0:QT])
                    else:
                        nc.sync.dma_start(
                            dyn(xts[cc][:, 0:QT], toff),
                            dyn(x_ext[cc][:, 0:QT], toff),
                        )

            if reps < 0:  # unrolled single-quarter bodies (sim analysis)
                for _ in range(-reps):
                    body(None)
            else:
                with tc.For_i(0, reps * NQ, 1, staggered_reset=staggered) as iv:
                    body((iv % NQ) * QT)

    nc.compile()
    return nc


def get_nc():
    if "nc" not in _COMPILED:
        _COMPILED["nc"] = _build()
    return _COMPILED["nc"]


def _prep_inputs(x, W, b):
    import ml_dtypes

    x = np.asarray(x, dtype=np.float32)
    W = np.asarray(W, dtype=np.float32)
    b = np.asarray(b, dtype=np.float32)

    xpad = np.zeros((B, C, T + PAD), dtype=np.float32)
    xpad[:, :, PAD:] = x
    xh = np.ascontiguousarray(xpad.reshape(B, N_CCHUNK, 128, XC)).astype(
        ml_dtypes.bfloat16
    )

    kern = W.reshape(H, C, K)
    wt = np.empty((N_HCHUNK, 128, N_MM * 128), dtype=ml_dtypes.bfloat16)
    for hj in range(N_HCHUNK):
        for k in range(K):
            for cc in range(N_CCHUNK):
                q = k * N_CCHUNK + cc
                wt[hj, :, q * 128 : (q + 1) * 128] = kern[
                    hj * 128 : (hj + 1) * 128, cc * 128 : (cc + 1) * 128, k
                ].T

    bias_mat = np.ascontiguousarray(b.reshape(N_HCHUNK, 128).T)
    return xh, wt, bias_mat


def _get_exec():
    """Build (once) a jitted shard_map executable over the 8 cores."""
    if "exec" in _COMPILED:
        return _COMPILED["exec"]

    import jax
    from jax.experimental.shard_map import shard_map
    from jax.sharding import Mesh, PartitionSpec

    nc = get_nc()
    bass2jax.install_neuronx_cc_hook()
    assert nc.dbg_addr is None
    partition_name = nc.partition_id_tensor.name if nc.partition_id_tensor else None

    in_names, out_names, out_avals, zero_outs = [], [], [], []
    for alloc in nc.m.functions[0].allocations:
        if not isinstance(alloc, mybir.MemoryLocationSet):
            continue
        name = alloc.memorylocations[0].name
        if alloc.kind == "ExternalInput":
            if name != partition_name:
                in_names.append(name)
        elif alloc.kind == "ExternalOutput":
            shape = tuple(alloc.tensor_shape)
            dtype = mybir.dt.np(alloc.dtype)
            out_names.append(name)
            out_avals.append(jax.core.ShapedArray(shape, dtype))
            zero_outs.append(np.zeros(shape, dtype))
    n_params = len(in_names)
    all_names = in_names + out_names
    if partition_name is not None:
        all_names = all_names + [partition_name]

    def _body(*args):
        operands = list(args)
        if partition_name is not None:
            operands.append(bass2jax.partition_id_tensor())
        outs = bass2jax._bass_exec_p.bind(
            *operands,
            out_avals=tuple(out_avals),
            in_names=tuple(all_names),
            out_names=tuple(out_names),
            lowering_input_output_aliases=(),
            sim_require_finite=True,
            sim_require_nnan=True,
            nc=nc,
        )
        return tuple(outs)

    devices = jax.devices()[:N_CORES]
    mesh = Mesh(np.asarray(devices), ("core",))
    n_args = n_params + len(out_names)
    sharded = jax.jit(
        shard_map(
            _body,
            mesh=mesh,
            in_specs=(PartitionSpec("core"),) * n_args,
            out_specs=(PartitionSpec("core"),) * len(out_names),
            check_rep=False,
        ),
        keep_unused=True,
    )
    _COMPILED["exec"] = (sharded, in_names, out_names, out_avals, zero_outs, mesh)
    return _COMPILED["exec"]


def _make_args(in_maps):
    sharded, in_names, out_names, out_avals, zero_outs, mesh = _get_exec()
    concat_in = [
        np.concatenate([np.asarray(in_maps[c][nm]) for c in range(N_CORES)], axis=0)
        for nm in in_names
    ]
    concat_zeros = [
        np.zeros((N_CORES * z.shape[0], *z.shape[1:]), z.dtype) for z in zero_outs
    ]
    return concat_in + concat_zeros


def _run(in_maps):
    sharded, in_names, out_names, out_avals, zero_outs, mesh = _get_exec()
    out_arrs = sharded(*_make_args(in_maps))
    return [
        {
            nm: np.asarray(out_arrs[i]).reshape(N_CORES, *out_avals[i].shape)[c]
            for i, nm in enumerate(out_names)
        }
        for c in range(N_CORES)
    ]


def make_in_maps(x, W, b):
    xh, wt, bias_mat = _prep_inputs(x, W, b)
    return [
        {"x": np.ascontiguousarray(xh[i]), "wt": wt, "bias": bias_mat}
        for i in range(N_CORES)
    ]


def kernel(x, W, b):
    results = _run(make_in_maps(x, W, b))
    return np.stack(
        [results[i]["out"].astype(np.float32) for i in range(N_CORES)], axis=0
    )
